# revision 25
# baseline (speedup 1.0000x reference)
"""Multi-head causal attention (B=2, S=2048, D=1024, H=16) on 8 TRN2 cores.

Sharding: tensor-parallel over heads (4 groups of 4 heads) x data-parallel
over batch (2), one (batch, head-group) pair per core.

Per core:
  - Q/K projections computed transposed (QT/KT: [c, tok], c = head-major
    projection column) so scores can run with head-dim as the contraction.
  - V projection computed in [tok, c] layout, augmented with a ones column
    per head so the attention-weight row sums (softmax denominators) fall
    out of the AV matmul for free.
  - scoresT[k, q] = KT_h.T-slice @ QT_h-slice (K=64 contraction), exp on
    ACT (no max subtraction: scores are O(1) by construction), causal
    masking via precomputed 0/1 tiles multiplied on the diagonal blocks.
    Far-diagonal blocks (dlt>=2) only exp/mask the live column range; the
    dead range is memset to zero.
  - AV^T accumulated over k-blocks in PSUM; normalized straight out of
    PSUM: reciprocal of the ones-row read in place, gpsimd-broadcast, one
    fused tensor_mul writes AVT. W_O row-parallel partial output written
    as outT [dout, tok], interleaved per q-tile as soon as AVT is ready.
  - PE warm-up: dummy matmuls issued at t=0 keep the HAM clock-gate busy
    while the initial DMAs land. Input/const DMAs are spread across the
    sync/vector/gpsimd queues so no engine's issue stream delays compute.
Host side: inputs are pre-transposed/pre-cast, partial outputs summed over
the 4 head-group cores per batch, V-bias and output bias folded into an
effective bias added at gather time (softmax rows sum to 1).
"""

import sys

if "/opt/trn_rl_repo" not in sys.path:
    sys.path.insert(0, "/opt/trn_rl_repo")

import numpy as np
import ml_dtypes

import concourse.bass as bass
import concourse.bacc as bacc
import concourse.tile as tile
from concourse import mybir
from concourse.bass_utils import run_bass_kernel_spmd

F32 = mybir.dt.float32
F32R = mybir.dt.float32r
BF16 = mybir.dt.bfloat16

P = 128
S = 2048          # sequence length
D = 1024          # model dim
C = 256           # projection columns per core (4 heads x 64)
HG = 4            # heads per core
DK = 64           # head dim
ND = 8            # d-blocks of 128 in D
NTOK = 16         # token blocks of 128
NQ = 4            # q tiles of 512
QW = 512


def build_attention_nc(causal: bool):
    nc = bacc.Bacc(None, target_bir_lowering=False)

    xq = nc.dram_tensor("xq", [D, S], BF16, kind="ExternalInput")
    xk = nc.dram_tensor("xk", [D, S], BF16, kind="ExternalInput")
    xv = nc.dram_tensor("xv", [D, S], BF16, kind="ExternalInput")
    wq = nc.dram_tensor("wq", [P, ND * C], BF16, kind="ExternalInput")
    wk = nc.dram_tensor("wk", [P, ND * C], BF16, kind="ExternalInput")
    wv = nc.dram_tensor("wv", [P, ND * C], BF16, kind="ExternalInput")
    wo = nc.dram_tensor("wo", [P, 2 * D], BF16, kind="ExternalInput")
    bq = nc.dram_tensor("bq", [P, 2], F32, kind="ExternalInput")
    bk = nc.dram_tensor("bk", [P, 2], F32, kind="ExternalInput")
    masks = nc.dram_tensor("masks", [P, 4 * QW], BF16, kind="ExternalInput")
    outT = nc.dram_tensor("outT", [D, S], BF16, kind="ExternalOutput")

    with tile.TileContext(nc) as tc:
        from contextlib import ExitStack

        with ExitStack() as ctx:
            const = ctx.enter_context(tc.tile_pool(name="const", bufs=1))
            xp = ctx.enter_context(tc.tile_pool(name="xp", bufs=24))
            resid = ctx.enter_context(tc.tile_pool(name="resid", bufs=1))
            epool = ctx.enter_context(tc.tile_pool(name="epool", bufs=16))
            dpool = ctx.enter_context(tc.tile_pool(name="dpool", bufs=5))
            opool = ctx.enter_context(tc.tile_pool(name="opool", bufs=8))
            ps_mm = ctx.enter_context(tc.tile_pool(name="ps_mm", bufs=2, space="PSUM"))
            ps_s = ctx.enter_context(tc.tile_pool(name="ps_s", bufs=2, space="PSUM"))
            ps_v = ctx.enter_context(tc.tile_pool(name="ps_v", bufs=2, space="PSUM"))

            # ---- PE warm-up: keep the HAM clock-gate busy while DMAs land.
            # Short N=128 matmuls (~110ns each cold) so the real first matmul
            # isn't delayed, but the PE activity window never goes idle.
            warm_w = const.tile([P, 2], BF16, name="warm_w")
            warm_x = const.tile([P, P], BF16, name="warm_x")
            nc.vector.memset(warm_w, 0.0)
            nc.vector.memset(warm_x, 0.0)
            warm_ps = ps_mm.tile([P, QW], F32, name="mm_ps")
            for _ in range(24):
                nc.tensor.matmul(warm_ps[0:2, 0:P], warm_w, warm_x,
                                 start=True, stop=True)

            # ---- constants ----
            wq_t = const.tile([P, ND, C], BF16, name="wq_t")
            wk_t = const.tile([P, ND, C], BF16, name="wk_t")
            wv_t = const.tile([P, ND, C], BF16, name="wv_t")
            bq_t = const.tile([P, 2], F32, name="bq_t")
            bk_t = const.tile([P, 2], F32, name="bk_t")
            wo_r = const.tile([P, 2, D], BF16, name="wo_r")
            # Each engine owns ONE hardware DMA ring (~65-110 GB/s, packet
            # size = per-partition row bytes), so ring assignment + order IS
            # the transfer schedule.  Weights are host-pre-tiled so each is
            # ONE DMA with 4KB rows.  gpsimd ring: weights then half of xk /
            # xv; scalar ring: bias + the other halves; sync: xq + th1 + out.
            nc.scalar.dma_start(out=bq_t, in_=bq[:, :])
            nc.gpsimd.dma_start(out=wq_t, in_=wq.rearrange("p (n c) -> p n c", n=ND))
            nc.gpsimd.dma_start(out=wk_t, in_=wk.rearrange("p (n c) -> p n c", n=ND))
            nc.gpsimd.dma_start(out=bk_t, in_=bk[:, :])
            if causal:
                mask_t = const.tile([P, 4, QW], BF16, name="mask_t")
                nc.gpsimd.dma_start(out=mask_t, in_=masks.rearrange("p (m f) -> p m f", m=4))

            # ---- residents ----
            QT = resid.tile([P, 2, S], BF16, name="QT")
            KT = resid.tile([P, 2, S], BF16, name="KT")
            Vp = resid.tile([P, NTOK, HG, DK + 1], BF16, name="Vp")
            AVT = resid.tile([P, 2, S], BF16, name="AVT")
            ones_t = const.tile([P, 1], BF16, name="ones_t")
            nc.vector.memset(ones_t, 1.0)
            nc.vector.tensor_copy(
                out=Vp[:, :, :, DK:DK + 1],
                in_=ones_t.to_broadcast((P, NTOK, HG, 1)),
            )

            # x loaded as [P, S/2] tiles (2KB DMA packets) in d-halves so
            # the two halves can ride different rings; key (which, th, dh).
            _x_cache = {}

            def get_x_half(which, th, dh, eng):
                key = (which, th, dh)
                if key not in _x_cache:
                    xdram = {"q": xq, "k": xk, "v": xv}[which]
                    xts = []
                    for d in range(4 * dh, 4 * dh + 4):
                        xt = xp.tile([P, S // 2], BF16, name="x_t")
                        eng.dma_start(
                            out=xt,
                            in_=xdram[d * P:(d + 1) * P,
                                      th * (S // 2):(th + 1) * (S // 2)],
                        )
                        xts.append(xt)
                    _x_cache[key] = xts
                return _x_cache[key]

            def x_tile(which, th, d):
                return _x_cache[(which, th, d // 4)][d % 4]

            def proj_qk_chunk(which, th, cs, t2):
                w_t, b_t, scale = (
                    (wq_t, bq_t, 0.125) if which == "q" else (wk_t, bk_t, 1.0)
                )
                dst = QT if which == "q" else KT
                ps = ps_mm.tile([P, QW], F32, name="mm_ps")
                for d in range(ND):
                    nc.tensor.matmul(
                        ps,
                        w_t[:, d, cs * P:(cs + 1) * P],
                        x_tile(which, th, d)[:, t2 * QW:(t2 + 1) * QW],
                        start=(d == 0),
                        stop=(d == ND - 1),
                        skip_group_check=True,
                    )
                # (psum * scale) + bias on DVE; ACT is reserved for exps
                nc.vector.tensor_scalar(
                    dst[:, cs, (th * 2 + t2) * QW:(th * 2 + t2 + 1) * QW],
                    ps,
                    scale,
                    b_t[:, cs:cs + 1],
                    op0=mybir.AluOpType.mult,
                    op1=mybir.AluOpType.add,
                )

            def emit_scores_unit(j, hp, kb):
                # Heads 2*hp (rows 0:64) and 2*hp+1 (rows 64:128) issue
                # back-to-back K=64 matmuls into the two halves (banks) of
                # one PSUM tile: disjoint row groups run concurrently in the
                # PE array, halving effective scores time.
                qs = slice(j * QW, (j + 1) * QW)
                sps = ps_s.tile([P, 2 * QW], F32, name="s_ps")
                for half in (0, 1):
                    rows = slice(half * DK, half * DK + DK)
                    nc.tensor.matmul(
                        sps[:, half * QW:(half + 1) * QW],
                        KT[rows, hp, kb * P:(kb + 1) * P],
                        QT[rows, hp, qs],
                        start=True,
                        stop=True,
                        skip_group_check=True,
                    )
                et = epool.tile([P, 2 * QW], BF16, name="e_t")
                dlt = kb - 4 * j
                if causal and dlt >= 2:
                    # dead columns [0, 128*dlt) per half: zero them and
                    # exp/mask only the live range.
                    w0 = P * dlt
                    for half in (0, 1):
                        h0 = half * QW
                        nc.vector.memset(et[:, h0:h0 + w0], 0.0)
                        nc.scalar.activation(
                            et[:, h0 + w0:h0 + QW],
                            sps[:, h0 + w0:h0 + QW],
                            mybir.ActivationFunctionType.Exp,
                        )
                        nc.vector.tensor_mul(
                            et[:, h0 + w0:h0 + QW],
                            et[:, h0 + w0:h0 + QW],
                            mask_t[:, dlt, w0:QW],
                        )
                else:
                    nc.scalar.activation(et, sps,
                                         mybir.ActivationFunctionType.Exp)
                    if causal and dlt >= 0:
                        for half in (0, 1):
                            nc.vector.tensor_mul(
                                et[:, half * QW:(half + 1) * QW],
                                et[:, half * QW:(half + 1) * QW],
                                mask_t[:, dlt, :],
                            )
                return et

            _av_state = {}

            def emit_avs_unit(j, hp, kb, nkb, et):
                qs = slice(j * QW, (j + 1) * QW)
                if kb == 0:
                    _av_state[(j, hp)] = {
                        0: ps_v.tile([P, QW], F32, name="av_ps"),
                        1: ps_v.tile([P, QW], F32, name="av_ps"),
                    }
                avps = _av_state[(j, hp)]
                for half in (0, 1):
                    nc.tensor.matmul(
                        avps[half][0:DK + 1, :],
                        Vp[:, kb, 2 * hp + half, :],
                        et[:, half * QW:(half + 1) * QW],
                        start=(kb == 0),
                        stop=(kb == nkb - 1),
                        skip_group_check=True,
                    )
                if kb == nkb - 1:
                    for half in (0, 1):
                        avp = avps[half]
                        # copy out of PSUM promptly (frees the bank for the
                        # next AV group), then normalize from SBUF.
                        avs = dpool.tile([DK, QW], F32, name="avs_t")
                        nc.vector.tensor_copy(out=avs, in_=avp[0:DK, :])
                        den = dpool.tile([1, QW], F32, name="den_t")
                        nc.vector.tensor_copy(out=den, in_=avp[DK:DK + 1, :])
                        rec = dpool.tile([1, QW], F32, name="rec_t")
                        nc.vector.reciprocal_approx_fast(out=rec, in_=den)
                        bc = dpool.tile([DK, QW], F32, name="bc_t")
                        nc.gpsimd.partition_broadcast(bc, rec)
                        nc.vector.tensor_mul(
                            AVT[half * DK:(half + 1) * DK, hp, qs],
                            avs,
                            bc,
                        )
                    del _av_state[(j, hp)]

            def proj_v_quantum(th, t8):
                ps = ps_mm.tile([P, QW], F32, name="mm_ps")
                for d in range(ND):
                    nc.tensor.matmul(
                        ps[:, 0:C],
                        x_tile("v", th, d)[:, t8 * P:(t8 + 1) * P],
                        wv_t[:, d, :],
                        start=(d == 0),
                        stop=(d == ND - 1),
                        skip_group_check=True,
                    )
                tok = th * 8 + t8
                nc.vector.tensor_copy(
                    out=Vp[:, tok, :, 0:DK],
                    in_=ps[:, 0:C].rearrange("p (h e) -> p h e", h=HG),
                )

            def final_quantum(qn, m):
                ps = ps_mm.tile([P, QW], F32, name="mm_ps")
                for cs in range(2):
                    nc.tensor.matmul(
                        ps,
                        wo_r[:, cs, m * P:(m + 1) * P],
                        AVT[:, cs, qn * QW:(qn + 1) * QW],
                        start=(cs == 0),
                        stop=(cs == 1),
                        skip_group_check=True,
                    )
                ot = opool.tile([P, QW], BF16, name="o_t")
                nc.vector.tensor_copy(out=ot, in_=ps)
                nc.sync.dma_start(
                    out=outT[m * P:(m + 1) * P, qn * QW:(qn + 1) * QW],
                    in_=ot,
                )

            # ---------------- software-pipelined schedule ----------------
            # The exp stream on ACT (~1.15us per k-block tile) is the clock
            # for the attention pipeline; PE's own work per tile (scores pair
            # + lagged AV pair) is only ~0.65us.  Emit everything in one
            # rotation that keeps the in-order PE queue stocked with ready
            # filler matmuls (projections, final output) whenever it would
            # otherwise head-of-line block on a scores->exp dependency.
            # LAG also gives the v-projection stream time to land its DMAs
            # before the first AV matmul would head-of-line block on Vp.
            LAG = 12

            def unit_list(jhps):
                out = []
                for j, hp in jhps:
                    nkb = 4 * j + 4 if causal else NTOK
                    for kb in range(nkb):
                        out.append((j, hp, kb, nkb))
                return out

            def exp_cost(j, kb):
                dlt = kb - 4 * j
                if causal and dlt >= 2:
                    return 2 * (QW - P * dlt + 352) / 1.2
                return (2 * QW + 352) / 1.2

            units = unit_list([(0, 0), (0, 1), (1, 0), (1, 1),
                               (2, 0), (2, 1), (3, 0), (3, 1)])
            uidx = {}
            for i, (j, hp, kb, nkb) in enumerate(units):
                uidx[(j, hp, kb)] = i

            # fillers: (ready_after, deadline, cost_ns, closure).
            # deadline = unit index before which this filler MUST be emitted
            # (it is a dependency of that unit); ready_after = unit index at
            # or after which it MAY be emitted (its own inputs are emitted).
            fillers = []

            def add(ra, dl, cost, fn):
                fillers.append((ra, dl, cost, fn))

            d_10 = uidx[(1, 0, 0)]
            d_20 = uidx[(2, 0, 0)]
            d_30 = uidx[(3, 0, 0)]
            # ordered roughly by deadline; ra on th=1 / v work reflects when
            # its input DMAs have landed on their ring.
            for t8 in range(4):
                add(1, uidx[(0, 0, t8)] + LAG, 856,
                    lambda t=t8: proj_v_quantum(0, t))
            for cs in range(2):
                add(0, d_10, 1707,
                    lambda c=cs: proj_qk_chunk("q", 0, c, 1))
            for cs in range(2):
                add(0, uidx[(1, 0, 4)], 1707,
                    lambda c=cs: proj_qk_chunk("k", 0, c, 1))
            for t8 in range(4, 8):
                add(4, uidx[(1, 0, t8)] + LAG, 856,
                    lambda t=t8: proj_v_quantum(0, t))
            # final(qn) only after avs(qn, 1) has been emitted (unit + LAG)
            f0 = uidx[(0, 1, 3)] + LAG + 1
            for m in range(ND):
                add(f0, 10**9, 427, lambda mm=m: final_quantum(0, mm))
            for cs in range(2):
                add(12, d_20, 1707,
                    lambda c=cs: proj_qk_chunk("q", 1, c, 0))
            for cs in range(2):
                add(16, uidx[(2, 0, 8)], 1707,
                    lambda c=cs: proj_qk_chunk("k", 1, c, 0))
            for cs in range(2):
                add(20, d_30, 1707,
                    lambda c=cs: proj_qk_chunk("q", 1, c, 1))
            for t8 in range(4):
                add(34, uidx[(2, 0, 8 + t8)] + LAG, 856,
                    lambda t=t8: proj_v_quantum(1, t))
            f1 = uidx[(1, 1, 7)] + LAG + 1
            for m in range(ND):
                add(f1, 10**9, 427, lambda mm=m: final_quantum(1, mm))
            for cs in range(2):
                add(30, uidx[(3, 0, 12)], 1707,
                    lambda c=cs: proj_qk_chunk("k", 1, c, 1))
            for t8 in range(4, 8):
                add(44, uidx[(3, 0, 8 + t8)] + 4, 856,
                    lambda t=t8: proj_v_quantum(1, t))
            f2 = uidx[(2, 1, 11)] + LAG + 1
            for m in range(ND):
                add(f2, 10**9, 427, lambda mm=m: final_quantum(2, mm))
            f3 = uidx[(3, 1, 15)] + LAG + 1
            for m in range(ND):
                add(f3, 10**9, 427, lambda mm=m: final_quantum(3, mm))

            # suffix-min of deadlines: forced flush must look past
            # no-deadline fillers sitting at the queue head.
            suf_dl = [10**9] * (len(fillers) + 1)
            for i in range(len(fillers) - 1, -1, -1):
                suf_dl[i] = min(fillers[i][1], suf_dl[i + 1])

            # ---- phase 0: ring-scheduled input prefetch + first q/k chunks.
            # sync: xq-th0 then th1 q/k then outT; gpsimd: weights (above)
            # then xk0 lower half, wv, xv0 lower, wo, th1 v; scalar: bq, the
            # upper halves of xk0/xv0 (its ring is slower but parallel).
            get_x_half("q", 0, 0, nc.sync)
            get_x_half("q", 0, 1, nc.sync)
            get_x_half("k", 0, 0, nc.gpsimd)
            get_x_half("k", 0, 1, nc.scalar)
            nc.gpsimd.dma_start(out=wv_t, in_=wv.rearrange("p (n c) -> p n c", n=ND))
            get_x_half("v", 0, 0, nc.gpsimd)
            get_x_half("v", 0, 1, nc.scalar)
            nc.gpsimd.dma_start(out=wo_r, in_=wo.rearrange("p (n d) -> p n d", n=2))
            get_x_half("q", 1, 0, nc.sync)
            get_x_half("q", 1, 1, nc.sync)
            get_x_half("k", 1, 0, nc.sync)
            get_x_half("k", 1, 1, nc.sync)
            get_x_half("v", 1, 0, nc.gpsimd)
            get_x_half("v", 1, 1, nc.gpsimd)
            proj_qk_chunk("k", 0, 0, 0)
            proj_qk_chunk("q", 0, 0, 0)
            proj_qk_chunk("k", 0, 1, 0)
            proj_qk_chunk("q", 0, 1, 0)

            # ---- rotation ----
            # Policy: the PE queue must stay OVERSTOCKED.  Before emitting
            # scores unit i (which back-pressures on exp(i-2) via the 2-deep
            # ps_s ring), pad the PE queue with fillers until the modeled PE
            # time reaches the modeled ACT completion of exp(i-2).  PE then
            # arrives at the scores matmul after its dependency resolved —
            # no micro-idle, no HAM re-throttle.
            A = []                       # cumulative exp completion estimate
            tot = 0.0
            for (j, hp, kb, nkb) in units:
                tot += exp_cost(j, kb)
                A.append(tot)
            pe_t = 0.0
            act_base = None
            fptr = 0
            pend = []                    # (unit_idx, et) awaiting AV emission

            def pop_fillers(now_idx, target):
                nonlocal fptr, pe_t
                while fptr < len(fillers):
                    ra, dl, cost, fn = fillers[fptr]
                    forced = suf_dl[fptr] <= now_idx
                    wanted = pe_t < target
                    if (forced or wanted) and ra <= now_idx:
                        fn()
                        pe_t += cost
                        fptr += 1
                    else:
                        break

            for i, (j, hp, kb, nkb) in enumerate(units):
                if act_base is None:
                    act_base = pe_t + 15000
                target = (act_base + A[i - 2] + 600) if i >= 2 else 0.0
                pop_fillers(i, target)
                et = emit_scores_unit(j, hp, kb)
                pe_t += 213
                pend.append((i, et))
                lag_i = LAG if i < 44 else 4
                while len(pend) > lag_i:
                    pi, pet = pend.pop(0)
                    pj, php, pkb, pnkb = units[pi]
                    emit_avs_unit(pj, php, pkb, pnkb, pet)
                    pe_t += 427
            for pi, pet in pend:
                pj, php, pkb, pnkb = units[pi]
                emit_avs_unit(pj, php, pkb, pnkb, pet)
            pop_fillers(10**9, 0.0)
            assert fptr == len(fillers)

    nc.compile()
    return nc


_NC_CACHE = {}


def _get_nc(causal: bool):
    if causal not in _NC_CACHE:
        _NC_CACHE[causal] = build_attention_nc(causal)
    return _NC_CACHE[causal]


def _causal_mask_tiles():
    # masks[delta][kk, qq] = 1.0 where (k0 + kk) <= (q0 + qq), k0 - q0 = 128*delta
    m = np.zeros((4, P, QW), np.float32)
    kk = np.arange(P)[:, None]
    qq = np.arange(QW)[None, :]
    for d in range(4):
        m[d] = (qq >= kk + d * P).astype(np.float32)
    return m


def _tile_w(WT_slice):
    # [D, C] (input-major) -> [P, ND*C] so the whole weight is one DMA with
    # 4KB contiguous rows per partition.
    return np.ascontiguousarray(
        WT_slice.reshape(ND, P, C).transpose(1, 0, 2).reshape(P, ND * C)
    ).astype(ml_dtypes.bfloat16)


def build_in_maps(query, key, value, Wq, bq, Wk, bk, Wv, Wo, causal):
    mask_tiles = (_causal_mask_tiles() if causal else np.ones((4, P, QW), np.float32))
    masks_h = np.ascontiguousarray(
        mask_tiles.transpose(1, 0, 2).reshape(P, 4 * QW)
    ).astype(ml_dtypes.bfloat16)

    xqT = [np.ascontiguousarray(query[b].T).astype(ml_dtypes.bfloat16) for b in range(2)]
    xkT = [np.ascontiguousarray(key[b].T).astype(ml_dtypes.bfloat16) for b in range(2)]
    xvT = [np.ascontiguousarray(value[b].T).astype(ml_dtypes.bfloat16) for b in range(2)]

    # torch Linear: y = x @ W.T; W.T is (in, out) = (d, c).
    WqT = np.ascontiguousarray(Wq.T)
    WkT = np.ascontiguousarray(Wk.T)
    WvT = np.ascontiguousarray(Wv.T)
    WoT = np.ascontiguousarray(Wo.T)

    in_maps = []
    for core in range(8):
        b, g = divmod(core, 4)
        cols = slice(g * C, (g + 1) * C)
        wo_h = np.ascontiguousarray(
            WoT[cols, :].reshape(2, P, D).transpose(1, 0, 2).reshape(P, 2 * D)
        ).astype(ml_dtypes.bfloat16)
        in_maps.append({
            "xq": xqT[b],
            "xk": xkT[b],
            "xv": xvT[b],
            "wq": _tile_w(np.ascontiguousarray(WqT[:, cols])),
            "wk": _tile_w(np.ascontiguousarray(WkT[:, cols])),
            "wv": _tile_w(np.ascontiguousarray(WvT[:, cols])),
            "wo": wo_h,
            "bq": np.ascontiguousarray((bq[cols] / 8.0).reshape(2, P).T),
            "bk": np.ascontiguousarray(bk[cols].reshape(2, P).T),
            "masks": masks_h,
        })
    return in_maps


def kernel(query, key, value, mask, Wq, bq, Wk, bk, Wv, bv, Wo, bo):
    query = np.asarray(query, np.float32)
    key = np.asarray(key, np.float32)
    value = np.asarray(value, np.float32)
    Wq = np.asarray(Wq, np.float32)
    Wk = np.asarray(Wk, np.float32)
    Wv = np.asarray(Wv, np.float32)
    Wo = np.asarray(Wo, np.float32)
    bq = np.asarray(bq, np.float32)
    bk = np.asarray(bk, np.float32)
    bv = np.asarray(bv, np.float32)
    bo = np.asarray(bo, np.float32)
    mask_np = np.asarray(mask)

    causal = bool(mask_np.any())
    if causal:
        idx = np.arange(S)
        expect = idx[None, :] > idx[:, None]
        if not np.array_equal(mask_np.reshape(S, S), expect):
            raise ValueError("kernel only supports the causal (or empty) mask")
    nc = _get_nc(causal)

    in_maps = build_in_maps(query, key, value, Wq, bq, Wk, bk, Wv, Wo, causal)

    res = run_bass_kernel_spmd(nc, in_maps, core_ids=list(range(8)))

    # softmax rows sum to 1, so the V bias contributes bv @ Wo.T to every row.
    bo_eff = bo + bv @ Wo.T
    out = np.empty((2, S, D), np.float32)
    for b in range(2):
        acc = res.results[b * 4]["outT"].astype(np.float32)
        for g in range(1, 4):
            acc += res.results[b * 4 + g]["outT"].astype(np.float32)
        out[b] = acc.T.astype(np.float32) + bo_eff
    return out


# revision 26
# speedup vs baseline: 1.0770x; 1.0770x over previous
"""Multi-head causal attention (B=2, S=2048, D=1024, H=16) on 8 TRN2 cores.

Sharding: tensor-parallel over heads (4 groups of 4 heads) x data-parallel
over batch (2), one (batch, head-group) pair per core.

Per core:
  - Q/K projections computed transposed (QT/KT: [c, tok], c = head-major
    projection column) so scores can run with head-dim as the contraction.
  - V projection computed in [tok, c] layout, augmented with a ones column
    per head so the attention-weight row sums (softmax denominators) fall
    out of the AV matmul for free.
  - scoresT[k, q] = KT_h.T-slice @ QT_h-slice (K=64 contraction), exp on
    ACT (no max subtraction: scores are O(1) by construction), causal
    masking via precomputed 0/1 tiles multiplied on the diagonal blocks.
  - AV^T accumulated over k-blocks in PSUM; normalized by the broadcast
    reciprocal of the ones-row; W_O row-parallel partial output written as
    outT [dout, tok].
Host side: inputs are pre-transposed/pre-cast, partial outputs summed over
the 4 head-group cores per batch, V-bias and output bias folded into an
effective bias added at gather time (softmax rows sum to 1).
"""

import sys

if "/opt/trn_rl_repo" not in sys.path:
    sys.path.insert(0, "/opt/trn_rl_repo")

import numpy as np
import ml_dtypes

import concourse.bass as bass
import concourse.bacc as bacc
import concourse.tile as tile
from concourse import mybir
from concourse.bass_utils import run_bass_kernel_spmd

F32 = mybir.dt.float32
F32R = mybir.dt.float32r
BF16 = mybir.dt.bfloat16

P = 128
S = 2048          # sequence length
D = 1024          # model dim
C = 256           # projection columns per core (4 heads x 64)
HG = 4            # heads per core
DK = 64           # head dim
ND = 8            # d-blocks of 128 in D
NTOK = 16         # token blocks of 128
NQ = 4            # q tiles of 512
QW = 512


def build_attention_nc(causal: bool):
    nc = bacc.Bacc(None, target_bir_lowering=False)

    xq = nc.dram_tensor("xq", [D, S], BF16, kind="ExternalInput")
    xk = nc.dram_tensor("xk", [D, S], BF16, kind="ExternalInput")
    xv = nc.dram_tensor("xv", [D, S], BF16, kind="ExternalInput")
    wq = nc.dram_tensor("wq", [D, C], BF16, kind="ExternalInput")
    wk = nc.dram_tensor("wk", [D, C], BF16, kind="ExternalInput")
    wv = nc.dram_tensor("wv", [D, C], BF16, kind="ExternalInput")
    wo = nc.dram_tensor("wo", [C, D], BF16, kind="ExternalInput")
    bq = nc.dram_tensor("bq", [P, 2], F32, kind="ExternalInput")
    bk = nc.dram_tensor("bk", [P, 2], F32, kind="ExternalInput")
    masks = nc.dram_tensor("masks", [4, P, QW], BF16, kind="ExternalInput")
    outT = nc.dram_tensor("outT", [D, S], BF16, kind="ExternalOutput")

    with tile.TileContext(nc) as tc:
        from contextlib import ExitStack

        with ExitStack() as ctx:
            const = ctx.enter_context(tc.tile_pool(name="const", bufs=1))
            xp = ctx.enter_context(tc.tile_pool(name="xp", bufs=12))
            resid = ctx.enter_context(tc.tile_pool(name="resid", bufs=1))
            epool = ctx.enter_context(tc.tile_pool(name="epool", bufs=34))
            dpool = ctx.enter_context(tc.tile_pool(name="dpool", bufs=4))
            opool = ctx.enter_context(tc.tile_pool(name="opool", bufs=8))
            ps_mm = ctx.enter_context(tc.tile_pool(name="ps_mm", bufs=3, space="PSUM"))
            ps_s = ctx.enter_context(tc.tile_pool(name="ps_s", bufs=2, space="PSUM"))
            ps_v = ctx.enter_context(tc.tile_pool(name="ps_v", bufs=1, space="PSUM"))

            # ---- constants ----
            wq_t = const.tile([P, ND, C], BF16, name="wq_t")
            wk_t = const.tile([P, ND, C], BF16, name="wk_t")
            wv_t = const.tile([P, ND, C], BF16, name="wv_t")
            for d in range(ND):
                nc.scalar.dma_start(out=wq_t[:, d, :], in_=wq[d * P:(d + 1) * P, :])
            for d in range(ND):
                nc.scalar.dma_start(out=wk_t[:, d, :], in_=wk[d * P:(d + 1) * P, :])
            for d in range(ND):
                nc.scalar.dma_start(out=wv_t[:, d, :], in_=wv[d * P:(d + 1) * P, :])
            wo_r = const.tile([P, 2, D], BF16, name="wo_r")
            nc.scalar.dma_start(out=wo_r, in_=wo.rearrange("(n p) d -> p n d", p=P))
            bq_t = const.tile([P, 2], F32, name="bq_t")
            bk_t = const.tile([P, 2], F32, name="bk_t")
            nc.scalar.dma_start(out=bq_t, in_=bq[:, :])
            nc.scalar.dma_start(out=bk_t, in_=bk[:, :])
            if causal:
                mask_t = const.tile([P, 4, QW], BF16, name="mask_t")
                nc.scalar.dma_start(out=mask_t, in_=masks.rearrange("m p f -> p m f"))

            # ---- residents ----
            QT = resid.tile([P, 2, S], BF16, name="QT")
            KT = resid.tile([P, 2, S], BF16, name="KT")
            Vp = resid.tile([P, NTOK, HG, DK + 1], BF16, name="Vp")
            AVT = resid.tile([P, 2, S], BF16, name="AVT")
            ones_t = const.tile([P, 1], BF16, name="ones_t")
            nc.vector.memset(ones_t, 1.0)
            nc.vector.tensor_copy(
                out=Vp[:, :, :, DK:DK + 1],
                in_=ones_t.to_broadcast((P, NTOK, HG, 1)),
            )

            def load_x_tiles(xdram, th):
                xts = []
                for d in range(ND):
                    xt = xp.tile([P, S // 2], BF16, name="x_t")
                    nc.sync.dma_start(
                        out=xt,
                        in_=xdram[d * P:(d + 1) * P, th * (S // 2):(th + 1) * (S // 2)],
                    )
                    xts.append(xt)
                return xts

            _x_cache = {}

            def get_x_tiles(which, th):
                if (which, th) not in _x_cache:
                    xdram = {"q": xq, "k": xk, "v": xv}[which]
                    _x_cache[(which, th)] = load_x_tiles(xdram, th)
                return _x_cache[(which, th)]

            def proj_qk_chunk(which, th, cs, t2):
                w_t, b_t, scale = (
                    (wq_t, bq_t, 0.125) if which == "q" else (wk_t, bk_t, 1.0)
                )
                dst = QT if which == "q" else KT
                xts = get_x_tiles(which, th)
                ps = ps_mm.tile([P, QW], F32, name="mm_ps")
                for d in range(ND):
                    nc.tensor.matmul(
                        ps,
                        w_t[:, d, cs * P:(cs + 1) * P],
                        xts[d][:, t2 * QW:(t2 + 1) * QW],
                        start=(d == 0),
                        stop=(d == ND - 1),
                    )
                # (psum * scale) + bias on DVE; ACT is reserved for exps
                nc.vector.tensor_scalar(
                    dst[:, cs, (th * 2 + t2) * QW:(th * 2 + t2 + 1) * QW],
                    ps,
                    scale,
                    b_t[:, cs:cs + 1],
                    op0=mybir.AluOpType.mult,
                    op1=mybir.AluOpType.add,
                )

            def proj_qk(which, th):
                for cs in range(2):
                    for t2 in range(2):
                        proj_qk_chunk(which, th, cs, t2)

            def proj_v(th):
                xts = get_x_tiles("v", th)
                for t8 in range(8):
                    ps = ps_mm.tile([P, QW], F32, name="mm_ps")
                    for d in range(ND):
                        nc.tensor.matmul(
                            ps[:, 0:C],
                            xts[d][:, t8 * P:(t8 + 1) * P],
                            wv_t[:, d, :],
                            start=(d == 0),
                            stop=(d == ND - 1),
                        )
                    tok = th * 8 + t8
                    nc.vector.tensor_copy(
                        out=Vp[:, tok, :, 0:DK],
                        in_=ps[:, 0:C].rearrange("p (h e) -> p h e", h=HG),
                    )

            def attn_scores(j, hp):
                # Heads 2*hp (rows 0:64) and 2*hp+1 (rows 64:128) issue
                # back-to-back K=64 matmuls into the two halves (banks) of
                # one PSUM tile: disjoint row groups run concurrently in the
                # PE array, halving effective scores time.
                nkb = 4 * j + 4 if causal else NTOK
                qs = slice(j * QW, (j + 1) * QW)
                ets = []
                for kb in range(nkb):
                    sps = ps_s.tile([P, 2 * QW], F32, name="s_ps")
                    for half in (0, 1):
                        rows = slice(half * DK, half * DK + DK)
                        nc.tensor.matmul(
                            sps[:, half * QW:(half + 1) * QW],
                            KT[rows, hp, kb * P:(kb + 1) * P],
                            QT[rows, hp, qs],
                            start=True,
                            stop=True,
                        )
                    et = epool.tile([P, 2 * QW], BF16, name="e_t")
                    nc.scalar.activation(et, sps, mybir.ActivationFunctionType.Exp)
                    if causal and kb >= 4 * j:
                        dlt = kb - 4 * j
                        for half in (0, 1):
                            nc.vector.tensor_mul(
                                et[:, half * QW:(half + 1) * QW],
                                et[:, half * QW:(half + 1) * QW],
                                mask_t[:, dlt, :],
                            )
                    ets.append(et)
                return ets

            def attn_avs(j, hp, ets):
                nkb = 4 * j + 4 if causal else NTOK
                qs = slice(j * QW, (j + 1) * QW)
                for half in (0, 1):
                    h = 2 * hp + half
                    hr = half
                    avp = ps_v.tile([P, QW], F32, name="av_ps")
                    for kb in range(nkb):
                        nc.tensor.matmul(
                            avp[0:DK + 1, :],
                            Vp[:, kb, h, :],
                            ets[kb][:, half * QW:(half + 1) * QW],
                            start=(kb == 0),
                            stop=(kb == nkb - 1),
                        )
                    avs = dpool.tile([DK, QW], F32, name="avs_t")
                    nc.vector.tensor_copy(out=avs, in_=avp[0:DK, :])
                    den = dpool.tile([1, QW], F32, name="den_t")
                    nc.vector.tensor_copy(out=den, in_=avp[DK:DK + 1, :])
                    rec = dpool.tile([1, QW], F32, name="rec_t")
                    nc.vector.reciprocal_approx_fast(out=rec, in_=den)
                    bc = dpool.tile([DK, QW], F32, name="bc_t")
                    nc.gpsimd.partition_broadcast(bc, rec)
                    nc.vector.tensor_mul(
                        AVT[hr * DK:(hr + 1) * DK, hp, qs],
                        avs,
                        bc,
                    )

            def final_proj(qn):
                for m in range(ND):
                    ps = ps_mm.tile([P, QW], F32, name="mm_ps")
                    for cs in range(2):
                        nc.tensor.matmul(
                            ps,
                            wo_r[:, cs, m * P:(m + 1) * P],
                            AVT[:, cs, qn * QW:(qn + 1) * QW],
                            start=(cs == 0),
                            stop=(cs == 1),
                        )
                    ot = opool.tile([P, QW], BF16, name="o_t")
                    nc.vector.tensor_copy(out=ot, in_=ps)
                    nc.sync.dma_start(
                        out=outT[m * P:(m + 1) * P, qn * QW:(qn + 1) * QW],
                        in_=ot,
                    )

            # Emission order interleaves projection halves with attention so
            # the big ACT exp load overlaps PE projection matmuls.
            proj_qk("q", 0)
            proj_qk("k", 0)
            e00 = attn_scores(0, 0)
            e01 = attn_scores(0, 1)
            e10 = attn_scores(1, 0)
            e11 = attn_scores(1, 1)
            proj_v(0)
            attn_avs(0, 0, e00)
            attn_avs(0, 1, e01)
            attn_avs(1, 0, e10)
            pending = (1, 1, e11)
            proj_qk("q", 1)
            proj_qk("k", 1)
            ets = attn_scores(2, 0)   # j=2 scores only need Q1/K1; fills the ACT bubble
            attn_avs(*pending)
            pending = (2, 0, ets)
            proj_v(1)
            for j, hp in ((2, 1), (3, 0), (3, 1)):
                ets = attn_scores(j, hp)
                attn_avs(*pending)
                pending = (j, hp, ets)
            attn_avs(*pending)
            for qn in range(NQ):
                final_proj(qn)

    nc.compile()
    return nc


_NC_CACHE = {}


def _get_nc(causal: bool):
    if causal not in _NC_CACHE:
        _NC_CACHE[causal] = build_attention_nc(causal)
    return _NC_CACHE[causal]


def _causal_mask_tiles():
    # masks[delta][kk, qq] = 1.0 where (k0 + kk) <= (q0 + qq), k0 - q0 = 128*delta
    m = np.zeros((4, P, QW), np.float32)
    kk = np.arange(P)[:, None]
    qq = np.arange(QW)[None, :]
    for d in range(4):
        m[d] = (qq >= kk + d * P).astype(np.float32)
    return m


def build_in_maps(query, key, value, Wq, bq, Wk, bk, Wv, Wo, causal):
    mask_tiles = (_causal_mask_tiles() if causal else np.ones((4, P, QW), np.float32)).astype(ml_dtypes.bfloat16)

    xqT = [np.ascontiguousarray(query[b].T).astype(ml_dtypes.bfloat16) for b in range(2)]
    xkT = [np.ascontiguousarray(key[b].T).astype(ml_dtypes.bfloat16) for b in range(2)]
    xvT = [np.ascontiguousarray(value[b].T).astype(ml_dtypes.bfloat16) for b in range(2)]

    # torch Linear: y = x @ W.T; W.T is (in, out) = (d, c).
    WqT = np.ascontiguousarray(Wq.T)
    WkT = np.ascontiguousarray(Wk.T)
    WvT = np.ascontiguousarray(Wv.T)
    WoT = np.ascontiguousarray(Wo.T)

    in_maps = []
    for core in range(8):
        b, g = divmod(core, 4)
        cols = slice(g * C, (g + 1) * C)
        in_maps.append({
            "xq": xqT[b],
            "xk": xkT[b],
            "xv": xvT[b],
            "wq": np.ascontiguousarray(WqT[:, cols]).astype(ml_dtypes.bfloat16),
            "wk": np.ascontiguousarray(WkT[:, cols]).astype(ml_dtypes.bfloat16),
            "wv": np.ascontiguousarray(WvT[:, cols]).astype(ml_dtypes.bfloat16),
            "wo": np.ascontiguousarray(WoT[cols, :]).astype(ml_dtypes.bfloat16),
            "bq": np.ascontiguousarray((bq[cols] / 8.0).reshape(2, P).T),
            "bk": np.ascontiguousarray(bk[cols].reshape(2, P).T),
            "masks": mask_tiles,
        })
    return in_maps


def kernel(query, key, value, mask, Wq, bq, Wk, bk, Wv, bv, Wo, bo):
    query = np.asarray(query, np.float32)
    key = np.asarray(key, np.float32)
    value = np.asarray(value, np.float32)
    Wq = np.asarray(Wq, np.float32)
    Wk = np.asarray(Wk, np.float32)
    Wv = np.asarray(Wv, np.float32)
    Wo = np.asarray(Wo, np.float32)
    bq = np.asarray(bq, np.float32)
    bk = np.asarray(bk, np.float32)
    bv = np.asarray(bv, np.float32)
    bo = np.asarray(bo, np.float32)
    mask_np = np.asarray(mask)

    causal = bool(mask_np.any())
    if causal:
        idx = np.arange(S)
        expect = idx[None, :] > idx[:, None]
        if not np.array_equal(mask_np.reshape(S, S), expect):
            raise ValueError("kernel only supports the causal (or empty) mask")
    nc = _get_nc(causal)

    in_maps = build_in_maps(query, key, value, Wq, bq, Wk, bk, Wv, Wo, causal)

    res = run_bass_kernel_spmd(nc, in_maps, core_ids=list(range(8)))

    # softmax rows sum to 1, so the V bias contributes bv @ Wo.T to every row.
    bo_eff = bo + bv @ Wo.T
    out = np.empty((2, S, D), np.float32)
    for b in range(2):
        acc = res.results[b * 4]["outT"].astype(np.float32)
        for g in range(1, 4):
            acc += res.results[b * 4 + g]["outT"].astype(np.float32)
        out[b] = acc.T.astype(np.float32) + bo_eff
    return out
